# revision 15
# baseline (speedup 1.0000x reference)
"""Trainium2 Bass kernel for nn_ClsCrossAttention (single-query CLS attention pooling).

Reference computation (per batch b, head h):
    tokens = features[b].reshape(C, H*W).T                  # [N=1024, C=768]
    K      = tokens @ W_k[h] + pos_embed                    # [N, 64]
    logits = K @ cls[h] / 8
    attn   = softmax(logits)
    out    = attn @ tokens                                  # [C]

Restructure (K is never materialized):
    logits[h, n] = tokens[n] . v_h + pos_bias[h, n]
        v_h      = W_k[h] @ cls[h] / 8          (host precomputed, [12, 768])
        pos_bias folded into the logits matmul as an extra K=64 contraction
        chunk with lhsT = (cls/8)^T, rhs = pos_embed^T.
    Logits are ~+-0.02 so softmax needs no max subtraction. With d = exp(l)-1
    (|d| <~ 0.05, so bf16 rounding of d is ~1e-5 absolute):
        out[h] = (colsum + d_h @ tokens) / (N + sum(d_h))
    where colsum = sum_n tokens[n] is computed exactly on the host in fp32 and
    broadcast-DMA'd; only the small correction d @ tokens runs in bf16 on the
    PE. Measured end-to-end L2 rel err ~7e-5.

Per core (8 of 64 batches), software-pipelined so the PE never waits on the
exp -> d -> dT chain or the PSUM->SBUF copy tail of the same batch:
    period b: [logits(b) | transposes(b) | dT(b-1) | pooling(b-1)]
    DMA   : features fp32 -> bf16 cast during DMA (SWDGE), 2x 1.5 MB per batch.
    PE    : 48x [128,128] bf16 transpose-mode matmuls (token-major layout),
            logits + pooling 2x column-tiled over the PE array with the two
            groups' matmuls interleaved for array-level concurrency.
    DVE/ACT: psum->sbuf copies split between the engines, exp, d=e-1,
            (pool+colsum)*recip(Z).
PSUM budget: transposes 3 + logits 2 + pooling 2 + dT 1 = 8 banks exactly.
"""

import sys

sys.path.insert(0, "/opt/trn_rl_repo")

import numpy as np
import ml_dtypes

import concourse.bass as bass
import concourse.mybir as mybir
from concourse import bacc
from concourse.tile import TileContext
from concourse.bass_utils import run_bass_kernel_spmd

BF16 = ml_dtypes.bfloat16

N_CORES = 8
B = 64
C = 768
N = 1024  # H*W = 32*32
NH = 12  # heads
DK = 64
BPC = B // N_CORES  # 8 batches per core
NCHUNK = C // 128  # 6 c-chunks
NTILE = N // 128  # 8 n-tiles
G = 2  # column-tile groups on the PE array
NHALF = N // G  # 512 logits columns per group
CHALF = C // G  # 384 output columns per group
# tokens_T layout: [c0..c383, ones, c384..c767, ones] -> 770 columns,
# each group's pooling rhs is a contiguous 385-column slice.
TOKW = C + G

_CACHE = {}


def _build_module():
    dt = mybir.dt
    nc = bacc.Bacc()

    feats = nc.dram_tensor("features", [BPC, C, N], dt.float32, kind="ExternalInput")
    colsum = nc.dram_tensor("colsum", [BPC, C], dt.float32, kind="ExternalInput")
    vT = nc.dram_tensor("vT", [128, NCHUNK, NH], dt.bfloat16, kind="ExternalInput")
    clsT = nc.dram_tensor("clsT", [DK, NH], dt.bfloat16, kind="ExternalInput")
    posT = nc.dram_tensor("posT", [DK, N], dt.bfloat16, kind="ExternalInput")
    ident = nc.dram_tensor("ident", [128, 128], dt.bfloat16, kind="ExternalInput")
    i12 = nc.dram_tensor("i12", [44, NH], dt.bfloat16, kind="ExternalInput")
    out = nc.dram_tensor("out", [BPC, NH, C], dt.float32, kind="ExternalOutput")

    with TileContext(nc) as tc:
        with (
            tc.tile_pool(name="consts", bufs=1) as consts,
            tc.tile_pool(name="xpool", bufs=3) as xpool,
            tc.tile_pool(name="tokpool", bufs=2) as tokpool,
            tc.tile_pool(name="sbmisc", bufs=2) as sbmisc,
            tc.tile_pool(name="tpsum", bufs=3, space="PSUM") as tpsum,
            tc.tile_pool(name="lpsum", bufs=1, space="PSUM") as lpsum,
            tc.tile_pool(name="ppsum", bufs=1, space="PSUM") as ppsum,
            tc.tile_pool(name="dtpsum", bufs=1, space="PSUM") as dtpsum,
        ):
            vT_sb = consts.tile([128, NCHUNK, NH], dt.bfloat16)
            nc.sync.dma_start(out=vT_sb, in_=vT[:])
            clsT_sb = consts.tile([DK, NH], dt.bfloat16)
            nc.sync.dma_start(out=clsT_sb, in_=clsT[:])
            posT_sb = consts.tile([DK, N], dt.bfloat16)
            nc.sync.dma_start(out=posT_sb, in_=posT[:])
            id_sb = consts.tile([128, 128], dt.bfloat16)
            nc.sync.dma_start(out=id_sb, in_=ident[:])
            i12_sb = consts.tile([44, NH], dt.bfloat16)
            nc.sync.dma_start(out=i12_sb, in_=i12[:])

            # colsum for all batches, broadcast to the 12 head rows of each
            # group's partition range, loaded once.
            cs_sb = consts.tile([44, BPC, CHALF], dt.float32)
            for g in range(G):
                src = colsum[:, g * CHALF : (g + 1) * CHALF]  # [BPC, 384]
                bcast = bass.AP(
                    tensor=src.tensor, offset=src.offset, ap=[[0, NH]] + src.ap
                )
                nc.gpsimd.dma_start(out=cs_sb[32 * g : 32 * g + NH, :, :], in_=bcast)

            state = {}  # per-batch tiles needed by the delayed (b-1) stages

            def emit_load(b):
                # fp32 via HWDGE (fast issue path); bf16 conversion for the
                # transpose path runs on the otherwise-idle GpSimd engine
                x_sb = xpool.tile([128, NCHUNK, N], dt.float32, name=f"x_{b}", tag="x")
                half = NCHUNK // 2
                for h in range(2):
                    ks = slice(h * half, (h + 1) * half)
                    nc.sync.dma_start(
                        out=x_sb[:, ks, :],
                        in_=feats[b].rearrange("(k p) n -> p k n", p=128)[:, ks, :],
                    )
                xb_sb = xpool.tile(
                    [128, NCHUNK, N], dt.bfloat16, name=f"xb_{b}", tag="xb"
                )
                for k in range(NCHUNK):
                    nc.gpsimd.tensor_copy(xb_sb[:, k, :], x_sb[:, k, :])
                return x_sb, xb_sb

            def emit_logits(b, x_sb):
                lps = []
                for g in range(G):
                    lps.append(
                        lpsum.tile(
                            [32 * g + NH, NHALF],
                            dt.float32,
                            name=f"lp{g}_{b}",
                            tag=f"lp{g}",
                        )
                    )
                # interleave the two column groups so the PE array runs both
                # concurrently (different col_grp strips). float32r runs the
                # fp32 moving operand at full rate (N=512 >= 256) and keeps
                # the logits path at fp32-grade precision.
                for k in range(NCHUNK):
                    for g in range(G):
                        lo = 32 * g
                        nc.tensor.matmul(
                            out=lps[g][lo : lo + NH, :],
                            lhsT=vT_sb[:, k, :],
                            rhs=x_sb[:, k, g * NHALF : (g + 1) * NHALF],
                            start=(k == 0),
                            stop=False,
                        )
                for g in range(G):
                    lo = 32 * g
                    nc.tensor.matmul(
                        out=lps[g][lo : lo + NH, :],
                        lhsT=clsT_sb[:],
                        rhs=posT_sb[:, g * NHALF : (g + 1) * NHALF],
                        start=False,
                        stop=True,
                    )
                return lps

            def emit_exp_d(b, lps):
                exp_sb = sbmisc.tile(
                    [44, NHALF], dt.float32, name=f"exp_{b}", tag="exp"
                )
                d_sb = sbmisc.tile([44, NHALF], dt.bfloat16, name=f"d_{b}", tag="d")
                for g in range(G):
                    lo = 32 * g
                    nc.scalar.activation(
                        out=exp_sb[lo : lo + NH, :],
                        in_=lps[g][lo : lo + NH, :],
                        func=mybir.ActivationFunctionType.Exp,
                    )
                    nc.vector.tensor_scalar_add(
                        d_sb[lo : lo + NH, :], exp_sb[lo : lo + NH, :], -1.0
                    )
                return d_sb

            def emit_tok_alloc(b):
                tok_sb = tokpool.tile(
                    [128, NTILE, TOKW], dt.bfloat16, name=f"tok_{b}", tag="tok"
                )
                nc.vector.memset(tok_sb[:, :, CHALF : CHALF + 1], 1.0)
                nc.vector.memset(tok_sb[:, :, TOKW - 1 : TOKW], 1.0)
                return tok_sb

            def emit_transpose_chunk(b, x_sb, tok_sb, k):
                tp = tpsum.tile(
                    [128, NTILE, 128], dt.bfloat16, name=f"tp_{b}_{k}", tag="tp"
                )
                for j in range(NTILE):
                    nc.tensor.transpose(
                        tp[:, j, :], x_sb[:, k, 128 * j : 128 * (j + 1)], id_sb[:]
                    )
                col = 128 * k if k < 3 else CHALF + 1 + 128 * (k - 3)
                dst = tok_sb[:, :, col : col + 128]
                if k % 2 == 0:
                    nc.vector.tensor_copy(dst, tp[:])
                else:
                    nc.scalar.copy(dst, tp[:])

            def emit_dT(b, d_sb):
                et = dtpsum.tile([128, NTILE, NH], dt.float32, name=f"et_{b}", tag="et")
                for j in range(NTILE):
                    g = j // 4
                    lo = 32 * g
                    jj = j % 4
                    nc.tensor.matmul(
                        out=et[:, j, :],
                        lhsT=d_sb[lo : lo + NH, 128 * jj : 128 * (jj + 1)],
                        rhs=i12_sb[lo : lo + NH, :],
                        start=True,
                        stop=True,
                    )
                dT_sb = sbmisc.tile(
                    [128, NTILE, NH], dt.bfloat16, name=f"dT_{b}", tag="dT"
                )
                nc.vector.tensor_copy(dT_sb[:], et[:])
                return dT_sb

            def emit_pool(b, dT_sb, tok_sb):
                pps = []
                for g in range(G):
                    pps.append(
                        ppsum.tile(
                            [32 * g + NH, CHALF + 1],
                            dt.float32,
                            name=f"pp{g}_{b}",
                            tag=f"pp{g}",
                        )
                    )
                for j in range(NTILE):
                    for g in range(G):
                        lo = 32 * g
                        nc.tensor.matmul(
                            out=pps[g][lo : lo + NH, :],
                            lhsT=dT_sb[:, j, :],
                            rhs=tok_sb[:, j, g * (CHALF + 1) : (g + 1) * (CHALF + 1)],
                            start=(j == 0),
                            stop=(j == NTILE - 1),
                        )
                for g in range(G):
                    lo = 32 * g
                    pp = pps[g]
                    zt = sbmisc.tile([44, 1], dt.float32, name=f"z{g}_{b}", tag=f"z{g}")
                    nc.vector.tensor_scalar_add(
                        zt[lo : lo + NH, :],
                        pp[lo : lo + NH, CHALF : CHALF + 1],
                        float(N),
                    )
                    recip = sbmisc.tile(
                        [44, 1], dt.float32, name=f"r{g}_{b}", tag=f"r{g}"
                    )
                    nc.vector.reciprocal(
                        out=recip[lo : lo + NH, :], in_=zt[lo : lo + NH, :]
                    )
                    num = sbmisc.tile(
                        [44, CHALF], dt.float32, name=f"n{g}_{b}", tag=f"n{g}"
                    )
                    nc.vector.tensor_add(
                        num[lo : lo + NH, :],
                        pp[lo : lo + NH, 0:CHALF],
                        cs_sb[lo : lo + NH, b, :],
                    )
                    osb = sbmisc.tile(
                        [44, CHALF], dt.float32, name=f"o{g}_{b}", tag=f"o{g}"
                    )
                    nc.vector.tensor_scalar_mul(
                        osb[lo : lo + NH, :],
                        num[lo : lo + NH, :],
                        recip[lo : lo + NH, :],
                    )
                    nc.sync.dma_start(
                        out=out[b, :, g * CHALF : (g + 1) * CHALF],
                        in_=osb[lo : lo + NH, :],
                    )

            for b in range(BPC):
                x_sb, xb_sb = emit_load(b)
                lps = emit_logits(b, xb_sb)
                d_sb = emit_exp_d(b, lps)
                tok_sb = emit_tok_alloc(b)
                for k in range(3):
                    emit_transpose_chunk(b, xb_sb, tok_sb, k)
                if b > 0:
                    # previous batch's dT between transpose bursts keeps the
                    # HAM activity monitor seeing normal matmul work
                    pdT = emit_dT(b - 1, state[b - 1]["d"])
                for k in range(3, NCHUNK):
                    emit_transpose_chunk(b, xb_sb, tok_sb, k)
                if b > 0:
                    emit_pool(b - 1, pdT, state[b - 1]["tok"])
                    del state[b - 1]
                state[b] = {"d": d_sb, "tok": tok_sb}

            bb = BPC - 1
            pdT = emit_dT(bb, state[bb]["d"])
            emit_pool(bb, pdT, state[bb]["tok"])

    nc.compile()
    return nc


def _host_consts(cls, W_k, pos_embed):
    # v_h = W_k[h] @ cls[h] / 8;  lhsT layout [128, chunk, head]
    V = np.einsum("hcd,hd->hc", W_k.astype(np.float32), cls.astype(np.float32)) / 8.0
    vT = np.ascontiguousarray(
        V.T.reshape(NCHUNK, 128, NH).transpose(1, 0, 2)
    )  # vT[p, k, h] = V[h, 128k+p]
    clsT = np.ascontiguousarray((cls / 8.0).T)  # [64, 12]
    posT = np.ascontiguousarray(pos_embed[0, 0].T)  # [64, 1024]
    ident = np.eye(128, dtype=np.float32)
    i12 = np.zeros((44, NH), np.float32)
    i12[0:NH, :] = np.eye(NH)
    i12[32 : 32 + NH, :] = np.eye(NH)
    return (
        vT.astype(BF16),
        clsT.astype(BF16),
        posT.astype(BF16),
        ident.astype(BF16),
        i12.astype(BF16),
    )


def kernel(features, cls, W_k, pos_embed):
    features = np.asarray(features, dtype=np.float32)
    cls = np.asarray(cls, dtype=np.float32)
    W_k = np.asarray(W_k, dtype=np.float32)
    pos_embed = np.asarray(pos_embed, dtype=np.float32)

    if "nc" not in _CACHE:
        _CACHE["nc"] = _build_module()
    nc = _CACHE["nc"]

    vT, clsT, posT, ident, i12 = _host_consts(cls, W_k, pos_embed)
    x = features.reshape(B, C, N)
    colsum = x.sum(axis=2, dtype=np.float64).astype(np.float32)  # [B, C] exact

    in_maps = []
    for core in range(N_CORES):
        sl = slice(core * BPC, (core + 1) * BPC)
        in_maps.append(
            {
                "features": np.ascontiguousarray(x[sl]),
                "colsum": np.ascontiguousarray(colsum[sl]),
                "vT": vT,
                "clsT": clsT,
                "posT": posT,
                "ident": ident,
                "i12": i12,
            }
        )

    res = run_bass_kernel_spmd(nc, in_maps, core_ids=list(range(N_CORES)))
    out = np.concatenate([r["out"] for r in res.results], axis=0)  # [64, 12, 768]
    return np.ascontiguousarray(out.reshape(B, NH * C)).astype(np.float32)


# revision 16
# speedup vs baseline: 1.8979x; 1.8979x over previous
"""Trainium2 Bass kernel for nn_ClsCrossAttention (single-query CLS attention pooling).

Reference computation (per batch b, head h):
    tokens = features[b].reshape(C, H*W).T                  # [N=1024, C=768]
    K      = tokens @ W_k[h] + pos_embed                    # [N, 64]
    logits = K @ cls[h] / 8
    attn   = softmax(logits)
    out    = attn @ tokens                                  # [C]

Restructure (K is never materialized):
    logits[h, n] = tokens[n] . v_h + pos_bias[h, n]
        v_h      = W_k[h] @ cls[h] / 8          (host precomputed, [12, 768])
        pos_bias folded into the logits matmul as an extra K=64 contraction
        chunk with lhsT = (cls/8)^T, rhs = pos_embed^T.
    Logits are ~+-0.02 so softmax needs no max subtraction. With d = exp(l)-1
    (|d| <~ 0.05, so bf16 rounding of d is ~1e-5 absolute):
        out[h] = (colsum + d_h @ tokens) / (N + sum(d_h))
    where colsum = sum_n tokens[n] is computed exactly on the host in fp32 and
    broadcast-DMA'd; only the small correction d @ tokens runs in bf16 on the
    PE. Measured end-to-end L2 rel err ~7e-5.

Per core (8 of 64 batches), software-pipelined so the PE never waits on the
exp -> d -> dT chain or the PSUM->SBUF copy tail of the same batch:
    period b: [logits(b) | transposes(b) | dT(b-1) | pooling(b-1)]
    DMA   : features fp32 -> bf16 cast during DMA (SWDGE), 2x 1.5 MB per batch.
    PE    : 48x [128,128] bf16 transpose-mode matmuls (token-major layout),
            logits + pooling 2x column-tiled over the PE array with the two
            groups' matmuls interleaved for array-level concurrency.
    DVE/ACT: psum->sbuf copies split between the engines, exp, d=e-1,
            (pool+colsum)*recip(Z).
PSUM budget: transposes 3 + logits 2 + pooling 2 + dT 1 = 8 banks exactly.
"""

import sys

sys.path.insert(0, "/opt/trn_rl_repo")

import numpy as np
import ml_dtypes

import concourse.bass as bass
import concourse.mybir as mybir
from concourse import bacc
from concourse.tile import TileContext
from concourse.bass_utils import run_bass_kernel_spmd

BF16 = ml_dtypes.bfloat16

N_CORES = 8
B = 64
C = 768
N = 1024  # H*W = 32*32
NH = 12  # heads
DK = 64
BPC = B // N_CORES  # 8 batches per core
NCHUNK = C // 128  # 6 c-chunks
NTILE = N // 128  # 8 n-tiles
G = 2  # column-tile groups on the PE array
NHALF = N // G  # 512 logits columns per group
CHALF = C // G  # 384 output columns per group
# tokens_T layout: [c0..c383, ones, c384..c767, ones] -> 770 columns,
# each group's pooling rhs is a contiguous 385-column slice.
TOKW = C + G

_CACHE = {}


def _build_module():
    dt = mybir.dt
    nc = bacc.Bacc()

    feats = nc.dram_tensor("features", [BPC, C, N], dt.float32, kind="ExternalInput")
    colsum = nc.dram_tensor("colsum", [BPC, C], dt.float32, kind="ExternalInput")
    vT = nc.dram_tensor("vT", [128, NCHUNK, NH], dt.bfloat16, kind="ExternalInput")
    clsT = nc.dram_tensor("clsT", [DK, NH], dt.bfloat16, kind="ExternalInput")
    posT = nc.dram_tensor("posT", [DK, N], dt.bfloat16, kind="ExternalInput")
    ident = nc.dram_tensor("ident", [128, 128], dt.bfloat16, kind="ExternalInput")
    i12 = nc.dram_tensor("i12", [44, NH], dt.bfloat16, kind="ExternalInput")
    out = nc.dram_tensor("out", [BPC, NH, C], dt.float32, kind="ExternalOutput")

    with TileContext(nc) as tc:
        with (
            tc.tile_pool(name="consts", bufs=1) as consts,
            tc.tile_pool(name="xpool", bufs=3) as xpool,
            tc.tile_pool(name="tokpool", bufs=2) as tokpool,
            tc.tile_pool(name="sbmisc", bufs=2) as sbmisc,
            tc.tile_pool(name="tpsum", bufs=3, space="PSUM") as tpsum,
            tc.tile_pool(name="lpsum", bufs=1, space="PSUM") as lpsum,
            tc.tile_pool(name="ppsum", bufs=1, space="PSUM") as ppsum,
            tc.tile_pool(name="dtpsum", bufs=1, space="PSUM") as dtpsum,
        ):
            vT_sb = consts.tile([128, NCHUNK, NH], dt.bfloat16)
            nc.sync.dma_start(out=vT_sb, in_=vT[:])
            clsT_sb = consts.tile([DK, NH], dt.bfloat16)
            nc.sync.dma_start(out=clsT_sb, in_=clsT[:])
            posT_sb = consts.tile([DK, N], dt.bfloat16)
            nc.sync.dma_start(out=posT_sb, in_=posT[:])
            id_sb = consts.tile([128, 128], dt.bfloat16)
            nc.sync.dma_start(out=id_sb, in_=ident[:])
            i12_sb = consts.tile([44, NH], dt.bfloat16)
            nc.sync.dma_start(out=i12_sb, in_=i12[:])

            # colsum for all batches, broadcast to the 12 head rows of each
            # group's partition range, loaded once (emitted after batch 0's
            # feature load so it doesn't block startup on the SWDGE queue).
            cs_sb = consts.tile([44, BPC, CHALF], dt.float32)

            def emit_colsum():
                for g in range(G):
                    s = colsum[:, g * CHALF : (g + 1) * CHALF]  # [BPC, 384]
                    bcast = bass.AP(
                        tensor=s.tensor, offset=s.offset, ap=[[0, NH]] + s.ap
                    )
                    nc.gpsimd.dma_start(
                        out=cs_sb[32 * g : 32 * g + NH, :, :], in_=bcast
                    )

            state = {}  # per-batch tiles needed by the delayed (b-1) stages

            def emit_load(b):
                # fp32 -> bf16 cast during the DMA (SWDGE), two halves so the
                # first logits/transposes can start at the half boundary
                x_sb = xpool.tile([128, NCHUNK, N], dt.bfloat16, name=f"x_{b}", tag="x")
                half = NCHUNK // 2
                for h in range(2):
                    ks = slice(h * half, (h + 1) * half)
                    nc.gpsimd.dma_start(
                        out=x_sb[:, ks, :],
                        in_=feats[b].rearrange("(k p) n -> p k n", p=128)[:, ks, :],
                    )
                return x_sb

            def emit_logits(b, x_sb):
                lps = []
                for g in range(G):
                    lps.append(
                        lpsum.tile(
                            [32 * g + NH, NHALF],
                            dt.float32,
                            name=f"lp{g}_{b}",
                            tag=f"lp{g}",
                        )
                    )
                # interleave the two column groups so the PE array runs both
                # concurrently (different col_grp strips). float32r runs the
                # fp32 moving operand at full rate (N=512 >= 256) and keeps
                # the logits path at fp32-grade precision.
                for k in range(NCHUNK):
                    for g in range(G):
                        lo = 32 * g
                        nc.tensor.matmul(
                            out=lps[g][lo : lo + NH, :],
                            lhsT=vT_sb[:, k, :],
                            rhs=x_sb[:, k, g * NHALF : (g + 1) * NHALF],
                            start=(k == 0),
                            stop=False,
                        )
                for g in range(G):
                    lo = 32 * g
                    nc.tensor.matmul(
                        out=lps[g][lo : lo + NH, :],
                        lhsT=clsT_sb[:],
                        rhs=posT_sb[:, g * NHALF : (g + 1) * NHALF],
                        start=False,
                        stop=True,
                    )
                return lps

            def emit_exp_d(b, lps):
                exp_sb = sbmisc.tile(
                    [44, NHALF], dt.float32, name=f"exp_{b}", tag="exp"
                )
                d_sb = sbmisc.tile([44, NHALF], dt.bfloat16, name=f"d_{b}", tag="d")
                for g in range(G):
                    lo = 32 * g
                    nc.scalar.activation(
                        out=exp_sb[lo : lo + NH, :],
                        in_=lps[g][lo : lo + NH, :],
                        func=mybir.ActivationFunctionType.Exp,
                    )
                    nc.vector.tensor_scalar_add(
                        d_sb[lo : lo + NH, :], exp_sb[lo : lo + NH, :], -1.0
                    )
                return d_sb

            def emit_tok_alloc(b):
                tok_sb = tokpool.tile(
                    [128, NTILE, TOKW], dt.bfloat16, name=f"tok_{b}", tag="tok"
                )
                nc.vector.memset(tok_sb[:, :, CHALF : CHALF + 1], 1.0)
                nc.vector.memset(tok_sb[:, :, TOKW - 1 : TOKW], 1.0)
                return tok_sb

            def emit_transpose_chunk(b, x_sb, tok_sb, k):
                tp = tpsum.tile(
                    [128, NTILE, 128], dt.bfloat16, name=f"tp_{b}_{k}", tag="tp"
                )
                for j in range(NTILE):
                    nc.tensor.transpose(
                        tp[:, j, :], x_sb[:, k, 128 * j : 128 * (j + 1)], id_sb[:]
                    )
                col = 128 * k if k < 3 else CHALF + 1 + 128 * (k - 3)
                dst = tok_sb[:, :, col : col + 128]
                if k % 2 == 0:
                    nc.vector.tensor_copy(dst, tp[:])
                else:
                    nc.scalar.copy(dst, tp[:])

            def emit_dT(b, d_sb):
                et = dtpsum.tile([128, NTILE, NH], dt.float32, name=f"et_{b}", tag="et")
                for j in range(NTILE):
                    g = j // 4
                    lo = 32 * g
                    jj = j % 4
                    nc.tensor.matmul(
                        out=et[:, j, :],
                        lhsT=d_sb[lo : lo + NH, 128 * jj : 128 * (jj + 1)],
                        rhs=i12_sb[lo : lo + NH, :],
                        start=True,
                        stop=True,
                    )
                dT_sb = sbmisc.tile(
                    [128, NTILE, NH], dt.bfloat16, name=f"dT_{b}", tag="dT"
                )
                nc.vector.tensor_copy(dT_sb[:], et[:])
                return dT_sb

            def emit_pool(b, dT_sb, tok_sb):
                pps = []
                for g in range(G):
                    pps.append(
                        ppsum.tile(
                            [32 * g + NH, CHALF + 1],
                            dt.float32,
                            name=f"pp{g}_{b}",
                            tag=f"pp{g}",
                        )
                    )
                for j in range(NTILE):
                    for g in range(G):
                        lo = 32 * g
                        nc.tensor.matmul(
                            out=pps[g][lo : lo + NH, :],
                            lhsT=dT_sb[:, j, :],
                            rhs=tok_sb[:, j, g * (CHALF + 1) : (g + 1) * (CHALF + 1)],
                            start=(j == 0),
                            stop=(j == NTILE - 1),
                        )
                for g in range(G):
                    lo = 32 * g
                    pp = pps[g]
                    zt = sbmisc.tile([44, 1], dt.float32, name=f"z{g}_{b}", tag=f"z{g}")
                    nc.vector.tensor_scalar_add(
                        zt[lo : lo + NH, :],
                        pp[lo : lo + NH, CHALF : CHALF + 1],
                        float(N),
                    )
                    recip = sbmisc.tile(
                        [44, 1], dt.float32, name=f"r{g}_{b}", tag=f"r{g}"
                    )
                    nc.vector.reciprocal(
                        out=recip[lo : lo + NH, :], in_=zt[lo : lo + NH, :]
                    )
                    num = sbmisc.tile(
                        [44, CHALF], dt.float32, name=f"n{g}_{b}", tag=f"n{g}"
                    )
                    nc.vector.tensor_add(
                        num[lo : lo + NH, :],
                        pp[lo : lo + NH, 0:CHALF],
                        cs_sb[lo : lo + NH, b, :],
                    )
                    osb = sbmisc.tile(
                        [44, CHALF], dt.float32, name=f"o{g}_{b}", tag=f"o{g}"
                    )
                    nc.vector.tensor_scalar_mul(
                        osb[lo : lo + NH, :],
                        num[lo : lo + NH, :],
                        recip[lo : lo + NH, :],
                    )
                    nc.sync.dma_start(
                        out=out[b, :, g * CHALF : (g + 1) * CHALF],
                        in_=osb[lo : lo + NH, :],
                    )

            for b in range(BPC):
                x_sb = emit_load(b)
                if b == 0:
                    emit_colsum()
                lps = emit_logits(b, x_sb)
                d_sb = emit_exp_d(b, lps)
                tok_sb = emit_tok_alloc(b)
                for k in range(3):
                    emit_transpose_chunk(b, x_sb, tok_sb, k)
                if b > 0:
                    # previous batch's dT between transpose bursts keeps the
                    # HAM activity monitor seeing normal matmul work
                    pdT = emit_dT(b - 1, state[b - 1]["d"])
                for k in range(3, NCHUNK):
                    emit_transpose_chunk(b, x_sb, tok_sb, k)
                if b > 0:
                    emit_pool(b - 1, pdT, state[b - 1]["tok"])
                    del state[b - 1]
                state[b] = {"d": d_sb, "tok": tok_sb}

            bb = BPC - 1
            pdT = emit_dT(bb, state[bb]["d"])
            emit_pool(bb, pdT, state[bb]["tok"])

    nc.compile()
    return nc


def _host_consts(cls, W_k, pos_embed):
    # v_h = W_k[h] @ cls[h] / 8;  lhsT layout [128, chunk, head]
    V = np.einsum("hcd,hd->hc", W_k.astype(np.float32), cls.astype(np.float32)) / 8.0
    vT = np.ascontiguousarray(
        V.T.reshape(NCHUNK, 128, NH).transpose(1, 0, 2)
    )  # vT[p, k, h] = V[h, 128k+p]
    clsT = np.ascontiguousarray((cls / 8.0).T)  # [64, 12]
    posT = np.ascontiguousarray(pos_embed[0, 0].T)  # [64, 1024]
    ident = np.eye(128, dtype=np.float32)
    i12 = np.zeros((44, NH), np.float32)
    i12[0:NH, :] = np.eye(NH)
    i12[32 : 32 + NH, :] = np.eye(NH)
    return (
        vT.astype(BF16),
        clsT.astype(BF16),
        posT.astype(BF16),
        ident.astype(BF16),
        i12.astype(BF16),
    )


def kernel(features, cls, W_k, pos_embed):
    features = np.asarray(features, dtype=np.float32)
    cls = np.asarray(cls, dtype=np.float32)
    W_k = np.asarray(W_k, dtype=np.float32)
    pos_embed = np.asarray(pos_embed, dtype=np.float32)

    if "nc" not in _CACHE:
        _CACHE["nc"] = _build_module()
    nc = _CACHE["nc"]

    vT, clsT, posT, ident, i12 = _host_consts(cls, W_k, pos_embed)
    x = features.reshape(B, C, N)
    colsum = x.sum(axis=2, dtype=np.float64).astype(np.float32)  # [B, C] exact

    in_maps = []
    for core in range(N_CORES):
        sl = slice(core * BPC, (core + 1) * BPC)
        in_maps.append(
            {
                "features": np.ascontiguousarray(x[sl]),
                "colsum": np.ascontiguousarray(colsum[sl]),
                "vT": vT,
                "clsT": clsT,
                "posT": posT,
                "ident": ident,
                "i12": i12,
            }
        )

    res = run_bass_kernel_spmd(nc, in_maps, core_ids=list(range(N_CORES)))
    out = np.concatenate([r["out"] for r in res.results], axis=0)  # [64, 12, 768]
    return np.ascontiguousarray(out.reshape(B, NH * C)).astype(np.float32)


# revision 17
# speedup vs baseline: 1.9100x; 1.0063x over previous
"""Trainium2 Bass kernel for nn_ClsCrossAttention (single-query CLS attention pooling).

Reference computation (per batch b, head h):
    tokens = features[b].reshape(C, H*W).T                  # [N=1024, C=768]
    K      = tokens @ W_k[h] + pos_embed                    # [N, 64]
    logits = K @ cls[h] / 8
    attn   = softmax(logits)
    out    = attn @ tokens                                  # [C]

Restructure (K is never materialized):
    logits[h, n] = tokens[n] . v_h + pos_bias[h, n]
        v_h      = W_k[h] @ cls[h] / 8          (host precomputed, [12, 768])
        pos_bias folded into the logits matmul as an extra K=64 contraction
        chunk with lhsT = (cls/8)^T, rhs = pos_embed^T.
    Logits are ~+-0.02 so softmax needs no max subtraction. With d = exp(l)-1
    (|d| <~ 0.05, so bf16 rounding of d is ~1e-5 absolute):
        out[h] = (colsum + d_h @ tokens) / (N + sum(d_h))
    where colsum = sum_n tokens[n] is computed exactly on the host in fp32 and
    broadcast-DMA'd; only the small correction d @ tokens runs in bf16 on the
    PE. Measured end-to-end L2 rel err ~7e-5.

Per core (8 of 64 batches), software-pipelined so the PE never waits on the
exp -> d -> dT chain or the PSUM->SBUF copy tail of the same batch:
    period b: [logits(b) | transposes(b) | dT(b-1) | pooling(b-1)]
    DMA   : features fp32 -> bf16 cast during DMA (SWDGE), 2x 1.5 MB per batch.
    PE    : 48x [128,128] bf16 transpose-mode matmuls (token-major layout),
            logits + pooling 2x column-tiled over the PE array with the two
            groups' matmuls interleaved for array-level concurrency.
    DVE/ACT: psum->sbuf copies split between the engines, exp, d=e-1,
            (pool+colsum)*recip(Z).
PSUM budget: transposes 3 + logits 2 + pooling 2 + dT 1 = 8 banks exactly.
"""

import sys

sys.path.insert(0, "/opt/trn_rl_repo")

import numpy as np
import ml_dtypes

import concourse.bass as bass
import concourse.mybir as mybir
from concourse import bacc
from concourse.tile import TileContext
from concourse.bass_utils import run_bass_kernel_spmd

BF16 = ml_dtypes.bfloat16

N_CORES = 8
B = 64
C = 768
N = 1024  # H*W = 32*32
NH = 12  # heads
DK = 64
BPC = B // N_CORES  # 8 batches per core
NCHUNK = C // 128  # 6 c-chunks
NTILE = N // 128  # 8 n-tiles
G = 2  # column-tile groups on the PE array
NHALF = N // G  # 512 logits columns per group
CHALF = C // G  # 384 output columns per group
# tokens_T layout: [c0..c383, ones, c384..c767, ones] -> 770 columns,
# each group's pooling rhs is a contiguous 385-column slice.
TOKW = C + G

_CACHE = {}


def _build_module():
    dt = mybir.dt
    nc = bacc.Bacc()

    feats = nc.dram_tensor("features", [BPC, C, N], dt.float32, kind="ExternalInput")
    colsum = nc.dram_tensor("colsum", [BPC, C], dt.float32, kind="ExternalInput")
    vT = nc.dram_tensor("vT", [128, NCHUNK, NH], dt.bfloat16, kind="ExternalInput")
    clsT = nc.dram_tensor("clsT", [DK, NH], dt.bfloat16, kind="ExternalInput")
    posT = nc.dram_tensor("posT", [DK, N], dt.bfloat16, kind="ExternalInput")
    ident = nc.dram_tensor("ident", [128, 128], dt.bfloat16, kind="ExternalInput")
    i12 = nc.dram_tensor("i12", [44, NH], dt.bfloat16, kind="ExternalInput")
    out = nc.dram_tensor("out", [BPC, NH, C], dt.float32, kind="ExternalOutput")

    with TileContext(nc) as tc:
        with (
            tc.tile_pool(name="consts", bufs=1) as consts,
            tc.tile_pool(name="xpool", bufs=3) as xpool,
            tc.tile_pool(name="tokpool", bufs=2) as tokpool,
            tc.tile_pool(name="sbmisc", bufs=2) as sbmisc,
            tc.tile_pool(name="tpsum", bufs=3, space="PSUM") as tpsum,
            tc.tile_pool(name="lpsum", bufs=1, space="PSUM") as lpsum,
            tc.tile_pool(name="ppsum", bufs=1, space="PSUM") as ppsum,
            tc.tile_pool(name="dtpsum", bufs=1, space="PSUM") as dtpsum,
        ):
            vT_sb = consts.tile([128, NCHUNK, NH], dt.bfloat16)
            nc.sync.dma_start(out=vT_sb, in_=vT[:])
            clsT_sb = consts.tile([DK, NH], dt.bfloat16)
            nc.sync.dma_start(out=clsT_sb, in_=clsT[:])
            posT_sb = consts.tile([DK, N], dt.bfloat16)
            nc.sync.dma_start(out=posT_sb, in_=posT[:])
            id_sb = consts.tile([128, 128], dt.bfloat16)
            nc.sync.dma_start(out=id_sb, in_=ident[:])
            i12_sb = consts.tile([44, NH], dt.bfloat16)
            nc.sync.dma_start(out=i12_sb, in_=i12[:])

            # colsum for all batches, broadcast to the 12 head rows of each
            # group's partition range, loaded once (emitted after batch 0's
            # feature load so it doesn't block startup on the SWDGE queue).
            cs_sb = consts.tile([44, BPC, CHALF], dt.float32)

            def emit_colsum():
                for g in range(G):
                    s = colsum[:, g * CHALF : (g + 1) * CHALF]  # [BPC, 384]
                    bcast = bass.AP(
                        tensor=s.tensor, offset=s.offset, ap=[[0, NH]] + s.ap
                    )
                    nc.gpsimd.dma_start(
                        out=cs_sb[32 * g : 32 * g + NH, :, :], in_=bcast
                    )

            state = {}  # per-batch tiles needed by the delayed (b-1) stages

            def emit_load(b):
                # fp32 -> bf16 cast during the DMA (SWDGE). Batch 0 loads per
                # chunk so the first logits matmul starts as early as possible;
                # later batches use one big DMA (SWDGE issue + drain on the Q7
                # is ~3us per dma_start, so fewer is better once pipelined).
                x_sb = xpool.tile([128, NCHUNK, N], dt.bfloat16, name=f"x_{b}", tag="x")
                src = feats[b].rearrange("(k p) n -> p k n", p=128)
                if b == 0:
                    for k in range(NCHUNK):
                        nc.gpsimd.dma_start(
                            out=x_sb[:, k : k + 1, :], in_=src[:, k : k + 1, :]
                        )
                else:
                    nc.gpsimd.dma_start(out=x_sb, in_=src)
                return x_sb

            def emit_logits(b, x_sb):
                lps = []
                for g in range(G):
                    lps.append(
                        lpsum.tile(
                            [32 * g + NH, NHALF],
                            dt.float32,
                            name=f"lp{g}_{b}",
                            tag=f"lp{g}",
                        )
                    )
                # interleave the two column groups so the PE array runs both
                # concurrently (different col_grp strips). float32r runs the
                # fp32 moving operand at full rate (N=512 >= 256) and keeps
                # the logits path at fp32-grade precision.
                for k in range(NCHUNK):
                    for g in range(G):
                        lo = 32 * g
                        nc.tensor.matmul(
                            out=lps[g][lo : lo + NH, :],
                            lhsT=vT_sb[:, k, :],
                            rhs=x_sb[:, k, g * NHALF : (g + 1) * NHALF],
                            start=(k == 0),
                            stop=False,
                        )
                for g in range(G):
                    lo = 32 * g
                    nc.tensor.matmul(
                        out=lps[g][lo : lo + NH, :],
                        lhsT=clsT_sb[:],
                        rhs=posT_sb[:, g * NHALF : (g + 1) * NHALF],
                        start=False,
                        stop=True,
                    )
                return lps

            def emit_exp_d(b, lps):
                exp_sb = sbmisc.tile(
                    [44, NHALF], dt.float32, name=f"exp_{b}", tag="exp"
                )
                d_sb = sbmisc.tile([44, NHALF], dt.bfloat16, name=f"d_{b}", tag="d")
                for g in range(G):
                    lo = 32 * g
                    nc.scalar.activation(
                        out=exp_sb[lo : lo + NH, :],
                        in_=lps[g][lo : lo + NH, :],
                        func=mybir.ActivationFunctionType.Exp,
                    )
                    nc.vector.tensor_scalar_add(
                        d_sb[lo : lo + NH, :], exp_sb[lo : lo + NH, :], -1.0
                    )
                return d_sb

            def emit_tok_alloc(b):
                tok_sb = tokpool.tile(
                    [128, NTILE, TOKW], dt.bfloat16, name=f"tok_{b}", tag="tok"
                )
                nc.vector.memset(tok_sb[:, :, CHALF : CHALF + 1], 1.0)
                nc.vector.memset(tok_sb[:, :, TOKW - 1 : TOKW], 1.0)
                return tok_sb

            def emit_transpose_chunk(b, x_sb, tok_sb, k):
                tp = tpsum.tile(
                    [128, NTILE, 128], dt.bfloat16, name=f"tp_{b}_{k}", tag="tp"
                )
                for j in range(NTILE):
                    nc.tensor.transpose(
                        tp[:, j, :], x_sb[:, k, 128 * j : 128 * (j + 1)], id_sb[:]
                    )
                col = 128 * k if k < 3 else CHALF + 1 + 128 * (k - 3)
                dst = tok_sb[:, :, col : col + 128]
                if k % 2 == 0:
                    nc.vector.tensor_copy(dst, tp[:])
                else:
                    nc.scalar.copy(dst, tp[:])

            def emit_dT(b, d_sb):
                et = dtpsum.tile([128, NTILE, NH], dt.float32, name=f"et_{b}", tag="et")
                for j in range(NTILE):
                    g = j // 4
                    lo = 32 * g
                    jj = j % 4
                    nc.tensor.matmul(
                        out=et[:, j, :],
                        lhsT=d_sb[lo : lo + NH, 128 * jj : 128 * (jj + 1)],
                        rhs=i12_sb[lo : lo + NH, :],
                        start=True,
                        stop=True,
                    )
                dT_sb = sbmisc.tile(
                    [128, NTILE, NH], dt.bfloat16, name=f"dT_{b}", tag="dT"
                )
                nc.vector.tensor_copy(dT_sb[:], et[:])
                return dT_sb

            def emit_pool(b, dT_sb, tok_sb):
                pps = []
                for g in range(G):
                    pps.append(
                        ppsum.tile(
                            [32 * g + NH, CHALF + 1],
                            dt.float32,
                            name=f"pp{g}_{b}",
                            tag=f"pp{g}",
                        )
                    )
                for j in range(NTILE):
                    for g in range(G):
                        lo = 32 * g
                        nc.tensor.matmul(
                            out=pps[g][lo : lo + NH, :],
                            lhsT=dT_sb[:, j, :],
                            rhs=tok_sb[:, j, g * (CHALF + 1) : (g + 1) * (CHALF + 1)],
                            start=(j == 0),
                            stop=(j == NTILE - 1),
                        )
                for g in range(G):
                    lo = 32 * g
                    pp = pps[g]
                    zt = sbmisc.tile([44, 1], dt.float32, name=f"z{g}_{b}", tag=f"z{g}")
                    nc.vector.tensor_scalar_add(
                        zt[lo : lo + NH, :],
                        pp[lo : lo + NH, CHALF : CHALF + 1],
                        float(N),
                    )
                    recip = sbmisc.tile(
                        [44, 1], dt.float32, name=f"r{g}_{b}", tag=f"r{g}"
                    )
                    nc.vector.reciprocal(
                        out=recip[lo : lo + NH, :], in_=zt[lo : lo + NH, :]
                    )
                    num = sbmisc.tile(
                        [44, CHALF], dt.float32, name=f"n{g}_{b}", tag=f"n{g}"
                    )
                    nc.vector.tensor_add(
                        num[lo : lo + NH, :],
                        pp[lo : lo + NH, 0:CHALF],
                        cs_sb[lo : lo + NH, b, :],
                    )
                    osb = sbmisc.tile(
                        [44, CHALF], dt.float32, name=f"o{g}_{b}", tag=f"o{g}"
                    )
                    nc.vector.tensor_scalar_mul(
                        osb[lo : lo + NH, :],
                        num[lo : lo + NH, :],
                        recip[lo : lo + NH, :],
                    )
                    nc.sync.dma_start(
                        out=out[b, :, g * CHALF : (g + 1) * CHALF],
                        in_=osb[lo : lo + NH, :],
                    )

            for b in range(BPC):
                x_sb = emit_load(b)
                if b == 0:
                    emit_colsum()
                lps = emit_logits(b, x_sb)
                d_sb = emit_exp_d(b, lps)
                tok_sb = emit_tok_alloc(b)
                for k in range(3):
                    emit_transpose_chunk(b, x_sb, tok_sb, k)
                if b > 0:
                    # previous batch's dT between transpose bursts keeps the
                    # HAM activity monitor seeing normal matmul work
                    pdT = emit_dT(b - 1, state[b - 1]["d"])
                for k in range(3, NCHUNK):
                    emit_transpose_chunk(b, x_sb, tok_sb, k)
                if b > 0:
                    emit_pool(b - 1, pdT, state[b - 1]["tok"])
                    del state[b - 1]
                state[b] = {"d": d_sb, "tok": tok_sb}

            bb = BPC - 1
            pdT = emit_dT(bb, state[bb]["d"])
            emit_pool(bb, pdT, state[bb]["tok"])

    nc.compile()
    return nc


def _host_consts(cls, W_k, pos_embed):
    # v_h = W_k[h] @ cls[h] / 8;  lhsT layout [128, chunk, head]
    V = np.einsum("hcd,hd->hc", W_k.astype(np.float32), cls.astype(np.float32)) / 8.0
    vT = np.ascontiguousarray(
        V.T.reshape(NCHUNK, 128, NH).transpose(1, 0, 2)
    )  # vT[p, k, h] = V[h, 128k+p]
    clsT = np.ascontiguousarray((cls / 8.0).T)  # [64, 12]
    posT = np.ascontiguousarray(pos_embed[0, 0].T)  # [64, 1024]
    ident = np.eye(128, dtype=np.float32)
    i12 = np.zeros((44, NH), np.float32)
    i12[0:NH, :] = np.eye(NH)
    i12[32 : 32 + NH, :] = np.eye(NH)
    return (
        vT.astype(BF16),
        clsT.astype(BF16),
        posT.astype(BF16),
        ident.astype(BF16),
        i12.astype(BF16),
    )


def kernel(features, cls, W_k, pos_embed):
    features = np.asarray(features, dtype=np.float32)
    cls = np.asarray(cls, dtype=np.float32)
    W_k = np.asarray(W_k, dtype=np.float32)
    pos_embed = np.asarray(pos_embed, dtype=np.float32)

    if "nc" not in _CACHE:
        _CACHE["nc"] = _build_module()
    nc = _CACHE["nc"]

    vT, clsT, posT, ident, i12 = _host_consts(cls, W_k, pos_embed)
    x = features.reshape(B, C, N)
    colsum = x.sum(axis=2, dtype=np.float64).astype(np.float32)  # [B, C] exact

    in_maps = []
    for core in range(N_CORES):
        sl = slice(core * BPC, (core + 1) * BPC)
        in_maps.append(
            {
                "features": np.ascontiguousarray(x[sl]),
                "colsum": np.ascontiguousarray(colsum[sl]),
                "vT": vT,
                "clsT": clsT,
                "posT": posT,
                "ident": ident,
                "i12": i12,
            }
        )

    res = run_bass_kernel_spmd(nc, in_maps, core_ids=list(range(N_CORES)))
    out = np.concatenate([r["out"] for r in res.results], axis=0)  # [64, 12, 768]
    return np.ascontiguousarray(out.reshape(B, NH * C)).astype(np.float32)


# revision 18
# speedup vs baseline: 1.9420x; 1.0167x over previous
"""Trainium2 Bass kernel for nn_ClsCrossAttention (single-query CLS attention pooling).

Reference computation (per batch b, head h):
    tokens = features[b].reshape(C, H*W).T                  # [N=1024, C=768]
    K      = tokens @ W_k[h] + pos_embed                    # [N, 64]
    logits = K @ cls[h] / 8
    attn   = softmax(logits)
    out    = attn @ tokens                                  # [C]

Restructure (K is never materialized):
    logits[h, n] = tokens[n] . v_h + pos_bias[h, n]
        v_h      = W_k[h] @ cls[h] / 8          (host precomputed, [12, 768])
        pos_bias folded into the logits matmul as an extra K=64 contraction
        chunk with lhsT = (cls/8)^T, rhs = pos_embed^T.
    Logits are ~+-0.02 so softmax needs no max subtraction. With d = exp(l)-1
    (|d| <~ 0.05, so bf16 rounding of d is ~1e-5 absolute):
        out[h] = (colsum + d_h @ tokens) / (N + sum(d_h))
    where colsum = sum_n tokens[n] is computed exactly on the host in fp32 and
    broadcast-DMA'd; only the small correction d @ tokens runs in bf16 on the
    PE. Measured end-to-end L2 rel err ~7e-5.

Per core (8 of 64 batches), software-pipelined so the PE never waits on the
exp -> d -> dT chain or the PSUM->SBUF copy tail of the same batch:
    period b: [logits(b) | transposes(b) | dT(b-1) | pooling(b-1)]
    DMA   : features fp32 -> bf16 cast during DMA (SWDGE), 2x 1.5 MB per batch.
    PE    : 48x [128,128] bf16 transpose-mode matmuls (token-major layout),
            logits + pooling 2x column-tiled over the PE array with the two
            groups' matmuls interleaved for array-level concurrency.
    DVE/ACT: psum->sbuf copies split between the engines, exp, d=e-1,
            (pool+colsum)*recip(Z).
PSUM budget: transposes 3 + logits 2 + pooling 2 + dT 1 = 8 banks exactly.
"""

import sys

sys.path.insert(0, "/opt/trn_rl_repo")

import numpy as np
import ml_dtypes

import concourse.bass as bass
import concourse.mybir as mybir
from concourse import bacc
from concourse.tile import TileContext
from concourse.bass_utils import run_bass_kernel_spmd

BF16 = ml_dtypes.bfloat16

N_CORES = 8
B = 64
C = 768
N = 1024  # H*W = 32*32
NH = 12  # heads
DK = 64
BPC = B // N_CORES  # 8 batches per core
NCHUNK = C // 128  # 6 c-chunks
NTILE = N // 128  # 8 n-tiles
G = 2  # column-tile groups on the PE array
NHALF = N // G  # 512 logits columns per group
CHALF = C // G  # 384 output columns per group
# tokens_T layout: [c0..c383, ones, c384..c767, ones] -> 770 columns,
# each group's pooling rhs is a contiguous 385-column slice.
TOKW = C + G

_CACHE = {}


def _build_module():
    dt = mybir.dt
    nc = bacc.Bacc()

    feats = nc.dram_tensor("features", [BPC, C, N], dt.float32, kind="ExternalInput")
    colsum = nc.dram_tensor("colsum", [BPC, C], dt.float32, kind="ExternalInput")
    vT = nc.dram_tensor("vT", [128, NCHUNK, NH], dt.bfloat16, kind="ExternalInput")
    clsT = nc.dram_tensor("clsT", [DK, NH], dt.bfloat16, kind="ExternalInput")
    posT = nc.dram_tensor("posT", [DK, N], dt.bfloat16, kind="ExternalInput")
    ident = nc.dram_tensor("ident", [128, 128], dt.bfloat16, kind="ExternalInput")
    i12 = nc.dram_tensor("i12", [44, NH], dt.bfloat16, kind="ExternalInput")
    out = nc.dram_tensor("out", [BPC, NH, C], dt.float32, kind="ExternalOutput")

    with TileContext(nc) as tc:
        with (
            tc.tile_pool(name="consts", bufs=1) as consts,
            tc.tile_pool(name="xpool", bufs=4) as xpool,
            tc.tile_pool(name="tokpool", bufs=2) as tokpool,
            tc.tile_pool(name="sbmisc", bufs=2) as sbmisc,
            tc.tile_pool(name="tpsum", bufs=3, space="PSUM") as tpsum,
            tc.tile_pool(name="lpsum", bufs=1, space="PSUM") as lpsum,
            tc.tile_pool(name="ppsum", bufs=1, space="PSUM") as ppsum,
            tc.tile_pool(name="dtpsum", bufs=1, space="PSUM") as dtpsum,
        ):
            vT_sb = consts.tile([128, NCHUNK, NH], dt.bfloat16)
            nc.sync.dma_start(out=vT_sb, in_=vT[:])
            clsT_sb = consts.tile([DK, NH], dt.bfloat16)
            nc.sync.dma_start(out=clsT_sb, in_=clsT[:])
            posT_sb = consts.tile([DK, N], dt.bfloat16)
            nc.sync.dma_start(out=posT_sb, in_=posT[:])
            id_sb = consts.tile([128, 128], dt.bfloat16)
            nc.sync.dma_start(out=id_sb, in_=ident[:])
            i12_sb = consts.tile([44, NH], dt.bfloat16)
            nc.sync.dma_start(out=i12_sb, in_=i12[:])

            # colsum for all batches, broadcast to the 12 head rows of each
            # group's partition range, loaded once (emitted after batch 0's
            # feature load so it doesn't block startup on the SWDGE queue).
            cs_sb = consts.tile([44, BPC, CHALF], dt.float32)

            def emit_colsum():
                for g in range(G):
                    s = colsum[:, g * CHALF : (g + 1) * CHALF]  # [BPC, 384]
                    bcast = bass.AP(
                        tensor=s.tensor, offset=s.offset, ap=[[0, NH]] + s.ap
                    )
                    nc.gpsimd.dma_start(
                        out=cs_sb[32 * g : 32 * g + NH, :, :], in_=bcast
                    )

            state = {}  # per-batch tiles needed by the delayed (b-1) stages

            def emit_load(b):
                # fp32 -> bf16 cast during the DMA (SWDGE). Batch 0 loads per
                # chunk so the first logits matmul starts as early as possible;
                # later batches use one big DMA (SWDGE issue + drain on the Q7
                # is ~3us per dma_start, so fewer is better once pipelined).
                x_sb = xpool.tile([128, NCHUNK, N], dt.bfloat16, name=f"x_{b}", tag="x")
                src = feats[b].rearrange("(k p) n -> p k n", p=128)
                if b == 0:
                    for k in range(NCHUNK):
                        nc.gpsimd.dma_start(
                            out=x_sb[:, k : k + 1, :], in_=src[:, k : k + 1, :]
                        )
                else:
                    nc.gpsimd.dma_start(out=x_sb, in_=src)
                return x_sb

            def emit_logits(b, x_sb):
                lps = []
                for g in range(G):
                    lps.append(
                        lpsum.tile(
                            [32 * g + NH, NHALF],
                            dt.float32,
                            name=f"lp{g}_{b}",
                            tag=f"lp{g}",
                        )
                    )
                # interleave the two column groups so the PE array runs both
                # concurrently (different col_grp strips). float32r runs the
                # fp32 moving operand at full rate (N=512 >= 256) and keeps
                # the logits path at fp32-grade precision.
                for k in range(NCHUNK):
                    for g in range(G):
                        lo = 32 * g
                        nc.tensor.matmul(
                            out=lps[g][lo : lo + NH, :],
                            lhsT=vT_sb[:, k, :],
                            rhs=x_sb[:, k, g * NHALF : (g + 1) * NHALF],
                            start=(k == 0),
                            stop=False,
                        )
                for g in range(G):
                    lo = 32 * g
                    nc.tensor.matmul(
                        out=lps[g][lo : lo + NH, :],
                        lhsT=clsT_sb[:],
                        rhs=posT_sb[:, g * NHALF : (g + 1) * NHALF],
                        start=False,
                        stop=True,
                    )
                return lps

            def emit_exp_d(b, lps):
                exp_sb = sbmisc.tile(
                    [44, NHALF], dt.float32, name=f"exp_{b}", tag="exp"
                )
                d_sb = sbmisc.tile([44, NHALF], dt.bfloat16, name=f"d_{b}", tag="d")
                for g in range(G):
                    lo = 32 * g
                    nc.scalar.activation(
                        out=exp_sb[lo : lo + NH, :],
                        in_=lps[g][lo : lo + NH, :],
                        func=mybir.ActivationFunctionType.Exp,
                    )
                    nc.vector.tensor_scalar_add(
                        d_sb[lo : lo + NH, :], exp_sb[lo : lo + NH, :], -1.0
                    )
                return d_sb

            def emit_tok_alloc(b):
                tok_sb = tokpool.tile(
                    [128, NTILE, TOKW], dt.bfloat16, name=f"tok_{b}", tag="tok"
                )
                nc.vector.memset(tok_sb[:, :, CHALF : CHALF + 1], 1.0)
                nc.vector.memset(tok_sb[:, :, TOKW - 1 : TOKW], 1.0)
                return tok_sb

            def emit_transpose_chunk(b, x_sb, tok_sb, k):
                tp = tpsum.tile(
                    [128, NTILE, 128], dt.bfloat16, name=f"tp_{b}_{k}", tag="tp"
                )
                for j in range(NTILE):
                    nc.tensor.transpose(
                        tp[:, j, :], x_sb[:, k, 128 * j : 128 * (j + 1)], id_sb[:]
                    )
                col = 128 * k if k < 3 else CHALF + 1 + 128 * (k - 3)
                dst = tok_sb[:, :, col : col + 128]
                if k % 2 == 0:
                    nc.vector.tensor_copy(dst, tp[:])
                else:
                    nc.scalar.copy(dst, tp[:])

            def emit_dT(b, d_sb):
                et = dtpsum.tile([128, NTILE, NH], dt.float32, name=f"et_{b}", tag="et")
                for j in range(NTILE):
                    g = j // 4
                    lo = 32 * g
                    jj = j % 4
                    nc.tensor.matmul(
                        out=et[:, j, :],
                        lhsT=d_sb[lo : lo + NH, 128 * jj : 128 * (jj + 1)],
                        rhs=i12_sb[lo : lo + NH, :],
                        start=True,
                        stop=True,
                    )
                dT_sb = sbmisc.tile(
                    [128, NTILE, NH], dt.bfloat16, name=f"dT_{b}", tag="dT"
                )
                nc.vector.tensor_copy(dT_sb[:], et[:])
                return dT_sb

            def emit_pool(b, dT_sb, tok_sb):
                pps = []
                for g in range(G):
                    pps.append(
                        ppsum.tile(
                            [32 * g + NH, CHALF + 1],
                            dt.float32,
                            name=f"pp{g}_{b}",
                            tag=f"pp{g}",
                        )
                    )
                for j in range(NTILE):
                    for g in range(G):
                        lo = 32 * g
                        nc.tensor.matmul(
                            out=pps[g][lo : lo + NH, :],
                            lhsT=dT_sb[:, j, :],
                            rhs=tok_sb[:, j, g * (CHALF + 1) : (g + 1) * (CHALF + 1)],
                            start=(j == 0),
                            stop=(j == NTILE - 1),
                        )
                for g in range(G):
                    lo = 32 * g
                    pp = pps[g]
                    zt = sbmisc.tile([44, 1], dt.float32, name=f"z{g}_{b}", tag=f"z{g}")
                    nc.vector.tensor_scalar_add(
                        zt[lo : lo + NH, :],
                        pp[lo : lo + NH, CHALF : CHALF + 1],
                        float(N),
                    )
                    recip = sbmisc.tile(
                        [44, 1], dt.float32, name=f"r{g}_{b}", tag=f"r{g}"
                    )
                    nc.vector.reciprocal(
                        out=recip[lo : lo + NH, :], in_=zt[lo : lo + NH, :]
                    )
                    num = sbmisc.tile(
                        [44, CHALF], dt.float32, name=f"n{g}_{b}", tag=f"n{g}"
                    )
                    nc.vector.tensor_add(
                        num[lo : lo + NH, :],
                        pp[lo : lo + NH, 0:CHALF],
                        cs_sb[lo : lo + NH, b, :],
                    )
                    osb = sbmisc.tile(
                        [44, CHALF], dt.float32, name=f"o{g}_{b}", tag=f"o{g}"
                    )
                    nc.vector.tensor_scalar_mul(
                        osb[lo : lo + NH, :],
                        num[lo : lo + NH, :],
                        recip[lo : lo + NH, :],
                    )
                    nc.sync.dma_start(
                        out=out[b, :, g * CHALF : (g + 1) * CHALF],
                        in_=osb[lo : lo + NH, :],
                    )

            for b in range(BPC):
                x_sb = emit_load(b)
                if b == 0:
                    emit_colsum()
                lps = emit_logits(b, x_sb)
                d_sb = emit_exp_d(b, lps)
                tok_sb = emit_tok_alloc(b)
                for k in range(3):
                    emit_transpose_chunk(b, x_sb, tok_sb, k)
                if b > 0:
                    # previous batch's dT between transpose bursts keeps the
                    # HAM activity monitor seeing normal matmul work
                    pdT = emit_dT(b - 1, state[b - 1]["d"])
                for k in range(3, NCHUNK):
                    emit_transpose_chunk(b, x_sb, tok_sb, k)
                if b > 0:
                    emit_pool(b - 1, pdT, state[b - 1]["tok"])
                    del state[b - 1]
                state[b] = {"d": d_sb, "tok": tok_sb}

            bb = BPC - 1
            pdT = emit_dT(bb, state[bb]["d"])
            emit_pool(bb, pdT, state[bb]["tok"])

    nc.compile()
    return nc


def _host_consts(cls, W_k, pos_embed):
    # v_h = W_k[h] @ cls[h] / 8;  lhsT layout [128, chunk, head]
    V = np.einsum("hcd,hd->hc", W_k.astype(np.float32), cls.astype(np.float32)) / 8.0
    vT = np.ascontiguousarray(
        V.T.reshape(NCHUNK, 128, NH).transpose(1, 0, 2)
    )  # vT[p, k, h] = V[h, 128k+p]
    clsT = np.ascontiguousarray((cls / 8.0).T)  # [64, 12]
    posT = np.ascontiguousarray(pos_embed[0, 0].T)  # [64, 1024]
    ident = np.eye(128, dtype=np.float32)
    i12 = np.zeros((44, NH), np.float32)
    i12[0:NH, :] = np.eye(NH)
    i12[32 : 32 + NH, :] = np.eye(NH)
    return (
        vT.astype(BF16),
        clsT.astype(BF16),
        posT.astype(BF16),
        ident.astype(BF16),
        i12.astype(BF16),
    )


def kernel(features, cls, W_k, pos_embed):
    features = np.asarray(features, dtype=np.float32)
    cls = np.asarray(cls, dtype=np.float32)
    W_k = np.asarray(W_k, dtype=np.float32)
    pos_embed = np.asarray(pos_embed, dtype=np.float32)

    if "nc" not in _CACHE:
        _CACHE["nc"] = _build_module()
    nc = _CACHE["nc"]

    vT, clsT, posT, ident, i12 = _host_consts(cls, W_k, pos_embed)
    x = features.reshape(B, C, N)
    colsum = x.sum(axis=2, dtype=np.float64).astype(np.float32)  # [B, C] exact

    in_maps = []
    for core in range(N_CORES):
        sl = slice(core * BPC, (core + 1) * BPC)
        in_maps.append(
            {
                "features": np.ascontiguousarray(x[sl]),
                "colsum": np.ascontiguousarray(colsum[sl]),
                "vT": vT,
                "clsT": clsT,
                "posT": posT,
                "ident": ident,
                "i12": i12,
            }
        )

    res = run_bass_kernel_spmd(nc, in_maps, core_ids=list(range(N_CORES)))
    out = np.concatenate([r["out"] for r in res.results], axis=0)  # [64, 12, 768]
    return np.ascontiguousarray(out.reshape(B, NH * C)).astype(np.float32)


# revision 19
# speedup vs baseline: 1.9569x; 1.0077x over previous
"""Trainium2 Bass kernel for nn_ClsCrossAttention (single-query CLS attention pooling).

Reference computation (per batch b, head h):
    tokens = features[b].reshape(C, H*W).T                  # [N=1024, C=768]
    K      = tokens @ W_k[h] + pos_embed                    # [N, 64]
    logits = K @ cls[h] / 8
    attn   = softmax(logits)
    out    = attn @ tokens                                  # [C]

Restructure (K is never materialized):
    logits[h, n] = tokens[n] . v_h + pos_bias[h, n]
        v_h      = W_k[h] @ cls[h] / 8          (host precomputed, [12, 768])
        pos_bias folded into the logits matmul as an extra K=64 contraction
        chunk with lhsT = (cls/8)^T, rhs = pos_embed^T.
    Logits are ~+-0.02 so softmax needs no max subtraction. With d = exp(l)-1
    (|d| <~ 0.05, so bf16 rounding of d is ~1e-5 absolute):
        out[h] = (colsum + d_h @ tokens) / (N + sum(d_h))
    where colsum = sum_n tokens[n] is computed exactly on the host in fp32 and
    broadcast-DMA'd; only the small correction d @ tokens runs in bf16 on the
    PE. Measured end-to-end L2 rel err ~7e-5.

Per core (8 of 64 batches), software-pipelined so the PE never waits on the
exp -> d -> dT chain or the PSUM->SBUF copy tail of the same batch:
    period b: [logits(b) | transposes(b) | dT(b-1) | pooling(b-1)]
    DMA   : features fp32 -> bf16 cast during DMA (SWDGE), 2x 1.5 MB per batch.
    PE    : 48x [128,128] bf16 transpose-mode matmuls (token-major layout),
            logits + pooling 2x column-tiled over the PE array with the two
            groups' matmuls interleaved for array-level concurrency.
    DVE/ACT: psum->sbuf copies split between the engines, exp, d=e-1,
            (pool+colsum)*recip(Z).
PSUM budget: transposes 3 + logits 2 + pooling 2 + dT 1 = 8 banks exactly.
"""

import sys

sys.path.insert(0, "/opt/trn_rl_repo")

import numpy as np
import ml_dtypes

import concourse.bass as bass
import concourse.mybir as mybir
from concourse import bacc
from concourse.tile import TileContext
from concourse.bass_utils import run_bass_kernel_spmd

BF16 = ml_dtypes.bfloat16

N_CORES = 8
B = 64
C = 768
N = 1024  # H*W = 32*32
NH = 12  # heads
DK = 64
BPC = B // N_CORES  # 8 batches per core
NCHUNK = C // 128  # 6 c-chunks
NTILE = N // 128  # 8 n-tiles
G = 2  # column-tile groups on the PE array
NHALF = N // G  # 512 logits columns per group
CHALF = C // G  # 384 output columns per group
# tokens_T layout: [c0..c383, ones, c384..c767, ones] -> 770 columns,
# each group's pooling rhs is a contiguous 385-column slice.
TOKW = C + G

_CACHE = {}


def _build_module():
    dt = mybir.dt
    nc = bacc.Bacc()

    feats = nc.dram_tensor("features", [BPC, C, N], dt.float32, kind="ExternalInput")
    colsum = nc.dram_tensor("colsum", [BPC, C], dt.float32, kind="ExternalInput")
    vT = nc.dram_tensor("vT", [128, NCHUNK, NH], dt.bfloat16, kind="ExternalInput")
    clsT = nc.dram_tensor("clsT", [DK, NH], dt.bfloat16, kind="ExternalInput")
    posT = nc.dram_tensor("posT", [DK, N], dt.bfloat16, kind="ExternalInput")
    ident = nc.dram_tensor("ident", [128, 128], dt.bfloat16, kind="ExternalInput")
    i12 = nc.dram_tensor("i12", [44, NH], dt.bfloat16, kind="ExternalInput")
    out = nc.dram_tensor("out", [BPC, NH, C], dt.float32, kind="ExternalOutput")

    with TileContext(nc) as tc:
        with (
            tc.tile_pool(name="consts", bufs=1) as consts,
            tc.tile_pool(name="xpool", bufs=4) as xpool,
            tc.tile_pool(name="tokpool", bufs=2) as tokpool,
            tc.tile_pool(name="sbmisc", bufs=2) as sbmisc,
            tc.tile_pool(name="tpsum", bufs=3, space="PSUM") as tpsum,
            tc.tile_pool(name="lpsum", bufs=1, space="PSUM") as lpsum,
            tc.tile_pool(name="ppsum", bufs=1, space="PSUM") as ppsum,
            tc.tile_pool(name="dtpsum", bufs=1, space="PSUM") as dtpsum,
        ):
            vT_sb = consts.tile([128, NCHUNK, NH], dt.bfloat16)
            nc.sync.dma_start(out=vT_sb, in_=vT[:])
            clsT_sb = consts.tile([DK, NH], dt.bfloat16)
            nc.sync.dma_start(out=clsT_sb, in_=clsT[:])
            posT_sb = consts.tile([DK, N], dt.bfloat16)
            nc.sync.dma_start(out=posT_sb, in_=posT[:])
            id_sb = consts.tile([128, 128], dt.bfloat16)
            nc.sync.dma_start(out=id_sb, in_=ident[:])
            i12_sb = consts.tile([44, NH], dt.bfloat16)
            nc.sync.dma_start(out=i12_sb, in_=i12[:])

            # colsum for all batches, broadcast to the 12 head rows of each
            # group's partition range, loaded once (emitted after batch 0's
            # feature load so it doesn't block startup on the SWDGE queue).
            cs_sb = consts.tile([44, BPC, CHALF], dt.float32)

            def emit_colsum():
                for g in range(G):
                    s = colsum[:, g * CHALF : (g + 1) * CHALF]  # [BPC, 384]
                    bcast = bass.AP(
                        tensor=s.tensor, offset=s.offset, ap=[[0, NH]] + s.ap
                    )
                    nc.gpsimd.dma_start(
                        out=cs_sb[32 * g : 32 * g + NH, :, :], in_=bcast
                    )

            state = {}  # per-batch tiles needed by the delayed (b-1) stages

            def emit_load(b):
                # fp32 -> bf16 cast during the DMA (SWDGE). Batch 0 loads per
                # chunk so the first logits matmul starts as early as possible;
                # later batches use one big DMA (SWDGE issue + drain on the Q7
                # is ~3us per dma_start, so fewer is better once pipelined).
                x_sb = xpool.tile([128, NCHUNK, N], dt.bfloat16, name=f"x_{b}", tag="x")
                src = feats[b].rearrange("(k p) n -> p k n", p=128)
                if b == 0:
                    for k in range(NCHUNK):
                        nc.gpsimd.dma_start(
                            out=x_sb[:, k : k + 1, :], in_=src[:, k : k + 1, :]
                        )
                else:
                    half = NCHUNK // 2
                    for h in range(2):
                        ks = slice(h * half, (h + 1) * half)
                        nc.gpsimd.dma_start(out=x_sb[:, ks, :], in_=src[:, ks, :])
                return x_sb

            def emit_logits(b, x_sb):
                lps = []
                for g in range(G):
                    lps.append(
                        lpsum.tile(
                            [32 * g + NH, NHALF],
                            dt.float32,
                            name=f"lp{g}_{b}",
                            tag=f"lp{g}",
                        )
                    )
                # interleave the two column groups so the PE array runs both
                # concurrently (different col_grp strips). float32r runs the
                # fp32 moving operand at full rate (N=512 >= 256) and keeps
                # the logits path at fp32-grade precision.
                for k in range(NCHUNK):
                    for g in range(G):
                        lo = 32 * g
                        nc.tensor.matmul(
                            out=lps[g][lo : lo + NH, :],
                            lhsT=vT_sb[:, k, :],
                            rhs=x_sb[:, k, g * NHALF : (g + 1) * NHALF],
                            start=(k == 0),
                            stop=False,
                        )
                for g in range(G):
                    lo = 32 * g
                    nc.tensor.matmul(
                        out=lps[g][lo : lo + NH, :],
                        lhsT=clsT_sb[:],
                        rhs=posT_sb[:, g * NHALF : (g + 1) * NHALF],
                        start=False,
                        stop=True,
                    )
                return lps

            def emit_exp_d(b, lps):
                exp_sb = sbmisc.tile(
                    [44, NHALF], dt.float32, name=f"exp_{b}", tag="exp"
                )
                d_sb = sbmisc.tile([44, NHALF], dt.bfloat16, name=f"d_{b}", tag="d")
                for g in range(G):
                    lo = 32 * g
                    nc.scalar.activation(
                        out=exp_sb[lo : lo + NH, :],
                        in_=lps[g][lo : lo + NH, :],
                        func=mybir.ActivationFunctionType.Exp,
                    )
                    nc.vector.tensor_scalar_add(
                        d_sb[lo : lo + NH, :], exp_sb[lo : lo + NH, :], -1.0
                    )
                return d_sb

            def emit_tok_alloc(b):
                tok_sb = tokpool.tile(
                    [128, NTILE, TOKW], dt.bfloat16, name=f"tok_{b}", tag="tok"
                )
                nc.vector.memset(tok_sb[:, :, CHALF : CHALF + 1], 1.0)
                nc.vector.memset(tok_sb[:, :, TOKW - 1 : TOKW], 1.0)
                return tok_sb

            def emit_transpose_chunk(b, x_sb, tok_sb, k):
                tp = tpsum.tile(
                    [128, NTILE, 128], dt.bfloat16, name=f"tp_{b}_{k}", tag="tp"
                )
                for j in range(NTILE):
                    nc.tensor.transpose(
                        tp[:, j, :], x_sb[:, k, 128 * j : 128 * (j + 1)], id_sb[:]
                    )
                col = 128 * k if k < 3 else CHALF + 1 + 128 * (k - 3)
                dst = tok_sb[:, :, col : col + 128]
                if k % 2 == 0:
                    nc.vector.tensor_copy(dst, tp[:])
                else:
                    nc.scalar.copy(dst, tp[:])

            def emit_dT(b, d_sb):
                et = dtpsum.tile([128, NTILE, NH], dt.float32, name=f"et_{b}", tag="et")
                for j in range(NTILE):
                    g = j // 4
                    lo = 32 * g
                    jj = j % 4
                    nc.tensor.matmul(
                        out=et[:, j, :],
                        lhsT=d_sb[lo : lo + NH, 128 * jj : 128 * (jj + 1)],
                        rhs=i12_sb[lo : lo + NH, :],
                        start=True,
                        stop=True,
                    )
                dT_sb = sbmisc.tile(
                    [128, NTILE, NH], dt.bfloat16, name=f"dT_{b}", tag="dT"
                )
                nc.vector.tensor_copy(dT_sb[:], et[:])
                return dT_sb

            def emit_pool(b, dT_sb, tok_sb):
                pps = []
                for g in range(G):
                    pps.append(
                        ppsum.tile(
                            [32 * g + NH, CHALF + 1],
                            dt.float32,
                            name=f"pp{g}_{b}",
                            tag=f"pp{g}",
                        )
                    )
                for j in range(NTILE):
                    for g in range(G):
                        lo = 32 * g
                        nc.tensor.matmul(
                            out=pps[g][lo : lo + NH, :],
                            lhsT=dT_sb[:, j, :],
                            rhs=tok_sb[:, j, g * (CHALF + 1) : (g + 1) * (CHALF + 1)],
                            start=(j == 0),
                            stop=(j == NTILE - 1),
                        )
                for g in range(G):
                    lo = 32 * g
                    pp = pps[g]
                    zt = sbmisc.tile([44, 1], dt.float32, name=f"z{g}_{b}", tag=f"z{g}")
                    nc.vector.tensor_scalar_add(
                        zt[lo : lo + NH, :],
                        pp[lo : lo + NH, CHALF : CHALF + 1],
                        float(N),
                    )
                    recip = sbmisc.tile(
                        [44, 1], dt.float32, name=f"r{g}_{b}", tag=f"r{g}"
                    )
                    nc.vector.reciprocal(
                        out=recip[lo : lo + NH, :], in_=zt[lo : lo + NH, :]
                    )
                    num = sbmisc.tile(
                        [44, CHALF], dt.float32, name=f"n{g}_{b}", tag=f"n{g}"
                    )
                    nc.vector.tensor_add(
                        num[lo : lo + NH, :],
                        pp[lo : lo + NH, 0:CHALF],
                        cs_sb[lo : lo + NH, b, :],
                    )
                    osb = sbmisc.tile(
                        [44, CHALF], dt.float32, name=f"o{g}_{b}", tag=f"o{g}"
                    )
                    nc.vector.tensor_scalar_mul(
                        osb[lo : lo + NH, :],
                        num[lo : lo + NH, :],
                        recip[lo : lo + NH, :],
                    )
                    nc.sync.dma_start(
                        out=out[b, :, g * CHALF : (g + 1) * CHALF],
                        in_=osb[lo : lo + NH, :],
                    )

            for b in range(BPC):
                x_sb = emit_load(b)
                if b == 0:
                    emit_colsum()
                lps = emit_logits(b, x_sb)
                d_sb = emit_exp_d(b, lps)
                tok_sb = emit_tok_alloc(b)
                for k in range(3):
                    emit_transpose_chunk(b, x_sb, tok_sb, k)
                if b > 0:
                    # previous batch's dT between transpose bursts keeps the
                    # HAM activity monitor seeing normal matmul work
                    pdT = emit_dT(b - 1, state[b - 1]["d"])
                for k in range(3, NCHUNK):
                    emit_transpose_chunk(b, x_sb, tok_sb, k)
                if b > 0:
                    emit_pool(b - 1, pdT, state[b - 1]["tok"])
                    del state[b - 1]
                state[b] = {"d": d_sb, "tok": tok_sb}

            bb = BPC - 1
            pdT = emit_dT(bb, state[bb]["d"])
            emit_pool(bb, pdT, state[bb]["tok"])

    nc.compile()
    return nc


def _host_consts(cls, W_k, pos_embed):
    # v_h = W_k[h] @ cls[h] / 8;  lhsT layout [128, chunk, head]
    V = np.einsum("hcd,hd->hc", W_k.astype(np.float32), cls.astype(np.float32)) / 8.0
    vT = np.ascontiguousarray(
        V.T.reshape(NCHUNK, 128, NH).transpose(1, 0, 2)
    )  # vT[p, k, h] = V[h, 128k+p]
    clsT = np.ascontiguousarray((cls / 8.0).T)  # [64, 12]
    posT = np.ascontiguousarray(pos_embed[0, 0].T)  # [64, 1024]
    ident = np.eye(128, dtype=np.float32)
    i12 = np.zeros((44, NH), np.float32)
    i12[0:NH, :] = np.eye(NH)
    i12[32 : 32 + NH, :] = np.eye(NH)
    return (
        vT.astype(BF16),
        clsT.astype(BF16),
        posT.astype(BF16),
        ident.astype(BF16),
        i12.astype(BF16),
    )


def kernel(features, cls, W_k, pos_embed):
    features = np.asarray(features, dtype=np.float32)
    cls = np.asarray(cls, dtype=np.float32)
    W_k = np.asarray(W_k, dtype=np.float32)
    pos_embed = np.asarray(pos_embed, dtype=np.float32)

    if "nc" not in _CACHE:
        _CACHE["nc"] = _build_module()
    nc = _CACHE["nc"]

    vT, clsT, posT, ident, i12 = _host_consts(cls, W_k, pos_embed)
    x = features.reshape(B, C, N)
    colsum = x.sum(axis=2, dtype=np.float64).astype(np.float32)  # [B, C] exact

    in_maps = []
    for core in range(N_CORES):
        sl = slice(core * BPC, (core + 1) * BPC)
        in_maps.append(
            {
                "features": np.ascontiguousarray(x[sl]),
                "colsum": np.ascontiguousarray(colsum[sl]),
                "vT": vT,
                "clsT": clsT,
                "posT": posT,
                "ident": ident,
                "i12": i12,
            }
        )

    res = run_bass_kernel_spmd(nc, in_maps, core_ids=list(range(N_CORES)))
    out = np.concatenate([r["out"] for r in res.results], axis=0)  # [64, 12, 768]
    return np.ascontiguousarray(out.reshape(B, NH * C)).astype(np.float32)
